# revision 3
# baseline (speedup 1.0000x reference)
"""GCL_skip_global distributed Trainium2 kernel (v2: aggregate-then-project).

Key algebraic restructure vs v1: segment_sum((h@wh)*ng) * ng  ==
(ng*A*ng @ h) @ wh  (row scalings and the sparse aggregation commute with
the dense right-projection).  So each core aggregates RAW bf16 features
(gathered locally from a replicated full copy -- replication is the chosen
sharding for h/s, like the weights) and only then projects the 6250
aggregated rows.  This removes both 51MB projected-feature AllGathers
(the v1 bottleneck: ncfw collectives run at ~62 GB/s).

Per 128-dst-node tile, fully fused on device:
  dma_gather raw source rows (lo/hi int16 halves) ->
  one-hot segment-sum matmuls into PSUM  (z = A_w @ h, w=ng[dst]*ng[src]
  folded host-side into the bf16 one-hot values) ->
  PE transpose z -> fused projection  zT_g@wh + zT_f@ws + mT@wm + bias
  (bias via a K=1 matmul) -> ReLU on the scalar engine -> direct store.
"""
import sys
sys.path.insert(0, '/opt/trn_rl_repo')
import numpy as np
from concourse import bass, mybir, bacc
import concourse.tile as tile
from concourse.masks import make_identity

F32 = mybir.dt.float32
BF16 = mybir.dt.bfloat16
I16 = mybir.dt.int16
import ml_dtypes
NP_BF16 = ml_dtypes.bfloat16

CORES = 8
N = 50000
D = 512
NPC = N // CORES           # 6250 dst nodes per core
NPAD = 6272                # 49*128
MT = NPAD // 128           # 49 dst tiles per core
KT = D // 128              # 4 feature chunks
NFULL = 50176              # 392*128 padded full node count (src rows)
HIB = 32768                # int16 gather index split point


# ---------------------------------------------------------------- host prep

def _wrap16(vals):
    """[n] int16 -> [128, n/16]: idx j at (j%16, j//16), replicated to 8 Q7 cores."""
    return np.tile(vals.reshape(-1, 16).T, (8, 1))


def _pack_graph(src, dst, scale):
    """Pack one graph's edges for all cores with a COMMON per-tile block shape.

    Edges are owned by the dst node's core; within a core, tile t covers dst
    nodes [t*128, (t+1)*128).  Each tile's slots are [lo blocks][hi blocks]
    where lo edges have src row < HIB (int16-addressable directly) and hi
    edges gather from a +HIB base.  Block counts BLO[t]/BHI[t] are the max
    over cores so all 8 cores share one program.

    Returns (BLO, BHI, per_core list of (il16, ih16, oh)).
    """
    src = np.asarray(src).astype(np.int64)
    dst = np.asarray(dst).astype(np.int64)
    scale = np.asarray(scale, np.float32).reshape(-1)
    w_all = (scale[dst] * scale[src]).astype(np.float32)

    per_core = []
    nlo_all = np.zeros((CORES, MT), np.int64)
    nhi_all = np.zeros((CORES, MT), np.int64)
    for c in range(CORES):
        sel = (dst >= c * NPC) & (dst < (c + 1) * NPC)
        d = dst[sel] - c * NPC
        s = src[sel]
        w = w_all[sel]
        t = d >> 7
        sd = d & 127
        hi = (s >= HIB).astype(np.int64)
        order = np.lexsort((hi, t))
        d, s, w, t, sd, hi = (x[order] for x in (d, s, w, t, sd, hi))
        g = t * 2 + hi
        cnt = np.bincount(g, minlength=MT * 2)
        start = np.concatenate([[0], np.cumsum(cnt)[:-1]])
        rank = np.arange(len(d)) - start[g]
        nlo_all[c] = cnt[0::2]
        nhi_all[c] = cnt[1::2]
        per_core.append((s, w, t, sd, hi, rank))

    BLO = -(-nlo_all.max(axis=0) // 128)       # ceil
    BHI = -(-nhi_all.max(axis=0) // 128)
    B = BLO + BHI
    OHOFF = np.concatenate([[0], np.cumsum(B)[:-1]])
    LOOFF = np.concatenate([[0], np.cumsum(BLO)[:-1]])
    HIOFF = np.concatenate([[0], np.cumsum(BHI)[:-1]])
    TOTB, TOTLO, TOTHI = int(B.sum()), int(BLO.sum()), int(BHI.sum())

    outs = []
    for c in range(CORES):
        s, w, t, sd, hi, rank = per_core[c]
        lo_m = hi == 0
        pos = np.where(lo_m, rank, BLO[t] * 128 + rank)
        blk = pos >> 7
        p = pos & 127
        oh = np.zeros((128, TOTB * 128), NP_BF16)
        col = (OHOFF[t] + blk) * 128 + sd
        oh[p, col] = w.astype(NP_BF16)
        il = np.zeros(max(TOTLO * 128, 16), np.int16)
        il[LOOFF[t[lo_m]] * 128 + pos[lo_m]] = s[lo_m].astype(np.int16)
        hi_m = ~lo_m
        ih = np.zeros(max(TOTHI * 128, 16), np.int16)
        ih[HIOFF[t[hi_m]] * 128 + (pos[hi_m] - BLO[t[hi_m]] * 128)] = \
            (s[hi_m] - HIB).astype(np.int16)
        outs.append((_wrap16(il), _wrap16(ih), oh))
    return tuple(int(x) for x in BLO), tuple(int(x) for x in BHI), outs


def prep_inputs(inp):
    """Full inputs -> (per-core input maps, structure key for build_nc)."""
    h, s, m = (np.asarray(inp[k], np.float32) for k in ('h', 's', 'm'))
    norm_g = np.asarray(inp['norm_g'], np.float32).reshape(-1)
    norm_f = np.asarray(inp['norm_f'], np.float32).reshape(-1)
    wh, ws, wm = (np.asarray(inp[k], np.float32) for k in ('wh', 'ws', 'wm'))
    bias = (np.asarray(inp['bh']) + np.asarray(inp['bs'])
            + np.asarray(inp['bm'])).astype(np.float32)

    def full_pad(x):  # [N, D] -> [NFULL, D] bf16 (replicated full copy)
        xp = np.zeros((NFULL, D), NP_BF16)
        xp[:N] = x.astype(NP_BF16)
        return xp

    hfull = full_pad(h)
    sfull = full_pad(s)

    def wr(wmat):  # [D, D] -> [128, KT*D]: wr[p, k*D+j] = w[k*128+p, j]
        return np.ascontiguousarray(
            wmat.reshape(KT, 128, D).transpose(1, 0, 2).reshape(
                128, KT * D).astype(NP_BF16))

    whr, wsr, wmr = wr(wh), wr(ws), wr(wm)
    biasrow = bias.reshape(1, D).astype(NP_BF16)

    BLO_G, BHI_G, packs_g = _pack_graph(inp['src_g'], inp['dst_g'], norm_g)
    BLO_F, BHI_F, packs_f = _pack_graph(inp['src_f'], inp['dst_f'], norm_f)

    in_maps = []
    for c in range(CORES):
        sl = slice(c * NPC, (c + 1) * NPC)
        # mT[t, p, ct*128+d] = m[t*128+d, ct*128+p]  (transposed per tile)
        mp = np.zeros((NPAD, D), np.float32)
        mp[:NPC] = m[sl]
        mT = np.ascontiguousarray(
            mp.reshape(MT, 128, KT, 128).transpose(0, 3, 2, 1).reshape(
                MT, 128, KT * 128).astype(NP_BF16))
        ilg, ihg, ohg = packs_g[c]
        ilf, ihf, ohf = packs_f[c]
        in_maps.append({
            'hfull': hfull, 'sfull': sfull, 'mT': mT,
            'whr': whr, 'wsr': wsr, 'wmr': wmr, 'biasrow': biasrow,
            'ilg': ilg, 'ihg': ihg, 'ohg': ohg,
            'ilf': ilf, 'ihf': ihf, 'ohf': ohf,
        })
    key = (BLO_G, BHI_G, BLO_F, BHI_F)
    return in_maps, key


# ---------------------------------------------------------------- device code

def build_nc(key, reps=1):
    BLO_G, BHI_G, BLO_F, BHI_F = key
    B_G = [a + b for a, b in zip(BLO_G, BHI_G)]
    B_F = [a + b for a, b in zip(BLO_F, BHI_F)]
    TOTB_G, TOTLO_G, TOTHI_G = sum(B_G), sum(BLO_G), sum(BHI_G)
    TOTB_F, TOTLO_F, TOTHI_F = sum(B_F), sum(BLO_F), sum(BHI_F)
    BMAX = max(max(B_G), max(B_F))

    nc = bacc.Bacc("TRN2", target_bir_lowering=False, debug=False)

    hfull = nc.dram_tensor("hfull", [NFULL, D], BF16, kind="ExternalInput")
    sfull = nc.dram_tensor("sfull", [NFULL, D], BF16, kind="ExternalInput")
    mTd = nc.dram_tensor("mT", [MT, 128, KT * 128], BF16, kind="ExternalInput")
    whr = nc.dram_tensor("whr", [128, KT * D], BF16, kind="ExternalInput")
    wsr = nc.dram_tensor("wsr", [128, KT * D], BF16, kind="ExternalInput")
    wmr = nc.dram_tensor("wmr", [128, KT * D], BF16, kind="ExternalInput")
    biasrow = nc.dram_tensor("biasrow", [1, D], BF16, kind="ExternalInput")
    ilg = nc.dram_tensor("ilg", [128, max(TOTLO_G * 8, 1)], I16, kind="ExternalInput")
    ihg = nc.dram_tensor("ihg", [128, max(TOTHI_G * 8, 1)], I16, kind="ExternalInput")
    ohg = nc.dram_tensor("ohg", [128, TOTB_G * 128], BF16, kind="ExternalInput")
    ilf = nc.dram_tensor("ilf", [128, max(TOTLO_F * 8, 1)], I16, kind="ExternalInput")
    ihf = nc.dram_tensor("ihf", [128, max(TOTHI_F * 8, 1)], I16, kind="ExternalInput")
    ohf = nc.dram_tensor("ohf", [128, TOTB_F * 128], BF16, kind="ExternalInput")
    out = nc.dram_tensor("out", [NPAD, D], F32, kind="ExternalOutput")

    with tile.TileContext(nc) as tc:
        with (
            tc.tile_pool(name="w", bufs=1) as wp,
            tc.tile_pool(name="oh", bufs=3) as op_,
            tc.tile_pool(name="gat", bufs=4) as gp,
            tc.tile_pool(name="z", bufs=2) as zp,
            tc.tile_pool(name="mtp", bufs=3) as lp,
            tc.tile_pool(name="fin", bufs=3) as fp,
            tc.tile_pool(name="psz", bufs=3, space="PSUM") as ps_z,
            tc.tile_pool(name="pst", bufs=2, space="PSUM") as ps_t,
            tc.tile_pool(name="pso", bufs=2, space="PSUM") as ps_o,
        ):
            # ---- one-time loads / consts
            wh_sb = wp.tile([128, KT * D], BF16, tag="wh")
            nc.sync.dma_start(out=wh_sb[:], in_=whr[:, :])
            ws_sb = wp.tile([128, KT * D], BF16, tag="ws")
            nc.sync.dma_start(out=ws_sb[:], in_=wsr[:, :])
            wm_sb = wp.tile([128, KT * D], BF16, tag="wm")
            nc.sync.dma_start(out=wm_sb[:], in_=wmr[:, :])
            bias_sb = wp.tile([1, D], BF16, tag="bias")
            nc.sync.dma_start(out=bias_sb[:], in_=biasrow[:, :])
            ilg_sb = wp.tile([128, max(TOTLO_G * 8, 1)], I16, tag="ilg")
            nc.sync.dma_start(out=ilg_sb[:], in_=ilg[:, :])
            ihg_sb = wp.tile([128, max(TOTHI_G * 8, 1)], I16, tag="ihg")
            nc.sync.dma_start(out=ihg_sb[:], in_=ihg[:, :])
            ilf_sb = wp.tile([128, max(TOTLO_F * 8, 1)], I16, tag="ilf")
            nc.sync.dma_start(out=ilf_sb[:], in_=ilf[:, :])
            ihf_sb = wp.tile([128, max(TOTHI_F * 8, 1)], I16, tag="ihf")
            nc.sync.dma_start(out=ihf_sb[:], in_=ihf[:, :])
            ident_sb = wp.tile([128, 128], BF16, tag="ident")
            make_identity(nc, ident_sb[:])
            ones_sb = wp.tile([1, 128], BF16, tag="ones")
            nc.gpsimd.memset(ones_sb[:], 1.0)

            lo_off_g = np.concatenate([[0], np.cumsum(BLO_G)[:-1]])
            hi_off_g = np.concatenate([[0], np.cumsum(BHI_G)[:-1]])
            oh_off_g = np.concatenate([[0], np.cumsum(B_G)[:-1]])
            lo_off_f = np.concatenate([[0], np.cumsum(BLO_F)[:-1]])
            hi_off_f = np.concatenate([[0], np.cumsum(BHI_F)[:-1]])
            oh_off_f = np.concatenate([[0], np.cumsum(B_F)[:-1]])

            def aggregate(t, blo_l, bhi_l, lo_off, hi_off, oh_off,
                          il_sb, ih_sb, oh_dram, feat_dram, ztag):
                """Gather + one-hot segment-sum + transpose for one (tile, graph).

                Returns zT in SBUF: [128(feat within chunk), KT*128(dst)] bf16.
                """
                blo, bhi = blo_l[t], bhi_l[t]
                b = blo + bhi
                zt_sb = zp.tile([128, D], BF16, tag=f"zt{ztag}")
                if b == 0:
                    nc.vector.memset(zt_sb[:], 0.0)
                    return zt_sb
                o = op_.tile([128, BMAX * 128], BF16, tag=f"oh{ztag}")
                nc.sync.dma_start(
                    out=o[:, :b * 128],
                    in_=oh_dram[:, int(oh_off[t]) * 128:(int(oh_off[t]) + b) * 128])
                g = gp.tile([128, BMAX, D], BF16, tag="g")
                if blo:
                    nc.gpsimd.dma_gather(
                        out_ap=g[:, 0:blo, :], in_ap=feat_dram.ap()[:, :],
                        idxs_ap=il_sb[:, int(lo_off[t]) * 8:(int(lo_off[t]) + blo) * 8],
                        num_idxs=blo * 128, num_idxs_reg=blo * 128, elem_size=D)
                if bhi:
                    nc.gpsimd.dma_gather(
                        out_ap=g[:, blo:b, :], in_ap=feat_dram.ap()[HIB:, :],
                        idxs_ap=ih_sb[:, int(hi_off[t]) * 8:(int(hi_off[t]) + bhi) * 8],
                        num_idxs=bhi * 128, num_idxs_reg=bhi * 128, elem_size=D)
                zps = ps_z.tile([128, D], F32)
                for bb in range(b):
                    nc.tensor.matmul(
                        out=zps[:], lhsT=o[:, bb * 128:(bb + 1) * 128],
                        rhs=g[:, bb, :], start=(bb == 0), stop=(bb == b - 1))
                z_sb = zp.tile([128, D], BF16, tag=f"z{ztag}")
                nc.vector.tensor_copy(z_sb[:], zps[:])
                ztps = ps_t.tile([128, D], BF16)
                for ct in range(KT):
                    nc.tensor.transpose(
                        ztps[:, ct * 128:(ct + 1) * 128],
                        z_sb[:, ct * 128:(ct + 1) * 128], ident_sb[:])
                nc.vector.tensor_copy(zt_sb[:], ztps[:])
                return zt_sb

            for _rep in range(reps):
                for t in range(MT):
                    ztg = aggregate(t, BLO_G, BHI_G, lo_off_g, hi_off_g,
                                    oh_off_g, ilg_sb, ihg_sb, ohg, hfull, "g")
                    ztf = aggregate(t, BLO_F, BHI_F, lo_off_f, hi_off_f,
                                    oh_off_f, ilf_sb, ihf_sb, ohf, sfull, "f")
                    mt_sb = lp.tile([128, KT * 128], BF16, tag="mt")
                    nc.sync.dma_start(out=mt_sb[:], in_=mTd[t, :, :])
                    po = ps_o.tile([128, D], F32)
                    for ct in range(KT):
                        nc.tensor.matmul(
                            out=po[:], lhsT=ztg[:, ct * 128:(ct + 1) * 128],
                            rhs=wh_sb[:, ct * D:(ct + 1) * D],
                            start=(ct == 0), stop=False)
                    for ct in range(KT):
                        nc.tensor.matmul(
                            out=po[:], lhsT=ztf[:, ct * 128:(ct + 1) * 128],
                            rhs=ws_sb[:, ct * D:(ct + 1) * D],
                            start=False, stop=False)
                    for ct in range(KT):
                        nc.tensor.matmul(
                            out=po[:], lhsT=mt_sb[:, ct * 128:(ct + 1) * 128],
                            rhs=wm_sb[:, ct * D:(ct + 1) * D],
                            start=False, stop=False)
                    nc.tensor.matmul(
                        out=po[:], lhsT=ones_sb[:, :], rhs=bias_sb[:, :],
                        start=False, stop=True)
                    o_sb = fp.tile([128, D], F32, tag="o")
                    nc.scalar.activation(
                        out=o_sb[:], in_=po[:],
                        func=mybir.ActivationFunctionType.Relu)
                    nc.sync.dma_start(
                        out=out[t * 128:(t + 1) * 128, :], in_=o_sb[:])

    nc.compile()
    return nc


def postprocess(results):
    return np.concatenate([results[k]["out"][:NPC] for k in range(CORES)], axis=0)


# ---------------------------------------------------------------- entry point

_NC_CACHE = {}


def _get_nc(key, reps=1):
    k = (key, reps)
    if k not in _NC_CACHE:
        _NC_CACHE[k] = build_nc(key, reps=reps)
    return _NC_CACHE[k]


def kernel(**inputs) -> np.ndarray:
    from concourse.bass_utils import run_bass_kernel_spmd
    in_maps, key = prep_inputs(inputs)
    nc = _get_nc(key)
    res = run_bass_kernel_spmd(nc, in_maps, core_ids=list(range(CORES)))
    return postprocess(res.results)


# ------------------------------------------------------- timing helper (test)

def _compile_jit(nc):
    """Mimic bass2jax.run_bass_via_pjrt but return a reusable jitted callable
    (no donation) so repeated dispatch can be timed."""
    import jax
    from jax.sharding import Mesh, PartitionSpec, NamedSharding
    from jax.experimental.shard_map import shard_map
    from concourse import bass2jax

    bass2jax.install_neuronx_cc_hook()
    in_names, out_names, out_avals, zero_outs = [], [], [], []
    for alloc in nc.m.functions[0].allocations:
        if not isinstance(alloc, mybir.MemoryLocationSet):
            continue
        name = alloc.memorylocations[0].name
        if alloc.kind == "ExternalInput":
            if name != "partition_id":
                in_names.append(name)
        elif alloc.kind == "ExternalOutput":
            out_names.append(name)
            shape = tuple(alloc.tensor_shape)
            dtype = mybir.dt.np(alloc.dtype)
            out_avals.append(jax.core.ShapedArray(shape, dtype))
            zero_outs.append(np.zeros(shape, dtype))
    n_params = len(in_names)
    all_names = in_names + out_names + ["partition_id"]

    def _body(*args):
        operands = list(args) + [bass2jax.partition_id_tensor()]
        outs = bass2jax._bass_exec_p.bind(
            *operands, out_avals=tuple(out_avals), in_names=tuple(all_names),
            out_names=tuple(out_names), lowering_input_output_aliases=(),
            sim_require_finite=True, sim_require_nnan=True, nc=nc)
        return tuple(outs)

    devices = jax.devices()[:CORES]
    mesh = Mesh(np.asarray(devices), ("core",))
    n_outs = len(out_names)
    in_specs = (PartitionSpec("core"),) * (n_params + n_outs)
    out_specs = (PartitionSpec("core"),) * n_outs
    fn = jax.jit(shard_map(_body, mesh=mesh, in_specs=in_specs,
                           out_specs=out_specs, check_rep=False), keep_unused=True)
    sharding = NamedSharding(mesh, PartitionSpec("core"))
    return fn, in_names, zero_outs, sharding


def _timed_min(fn, args, n=12):
    import jax, time
    o = fn(*args)
    jax.block_until_ready(o)
    ts = []
    for _ in range(n):
        t0 = time.perf_counter()
        o = fn(*args)
        jax.block_until_ready(o)
        ts.append(time.perf_counter() - t0)
    return float(np.min(ts))


def _timed_nc(nc, in_maps, n=12):
    import jax
    fn, in_names, zero_outs, sh = _compile_jit(nc)
    args = [jax.device_put(
        np.concatenate([m[nm] for m in in_maps], axis=0), sh) for nm in in_names]
    args += [jax.device_put(
        np.zeros((CORES * z.shape[0], *z.shape[1:]), z.dtype), sh) for z in zero_outs]
    return _timed_min(fn, args, n)


def measure_hw_ns(inputs, n=12, reps_hi=3):
    """HW kernel time via reps differential: (wall(reps_hi) - wall(1))/(reps_hi-1).

    Cancels the constant axon dispatch overhead; validated against the
    grading harness on the v1 kernel (1931us local vs 1945us harness).
    """
    in_maps, key = prep_inputs(inputs)
    t1 = _timed_nc(_get_nc(key, reps=1), in_maps, n)
    th = _timed_nc(_get_nc(key, reps=reps_hi), in_maps, n)
    print(f"  [wall: reps=1 {t1*1e3:.2f} ms, reps={reps_hi} {th*1e3:.2f} ms]")
    return max(th - t1, 0.0) / (reps_hi - 1) * 1e9
